# revision 21
# baseline (speedup 1.0000x reference)
"""Trainium2 Bass kernel for CustomConvWithExtra.

out = conv3x3(x, w_main) + b_main + extra, where extra collapses to a 3x3
border-class table T[b,c,clsh,clsw] (conv of a spatially-constant image).

Design v3 (from 183us baseline; DMA packet-latency is the binding law):
 - Data parallel: 1 batch image per NeuronCore (B=8 = 8 cores).
 - Empirical DMA law (measured): ANY packet that WRITES SBUF costs ~1.05us
   flat (HBM round-trip latency, no pipelining, packet <= 16KB); only
   SBUF->HBM writes stream at ~26GB/s/engine.  So the input path must
   minimize SBUF-write packet count and the kw=0/2 duplicate planes must
   NOT go through DMA at all.
 - Supertile = 32 output row-pairs; 8 supertiles.  Patch tile [88, 8192]
   bf16: each kw plane holds 24 rows = (d,ci) x (g=half), each row 16
   pair-segments of 512 = 16KB -> fill is 24 descriptors of EXACTLY 16KB
   (one full packet each; 3.2MB total input = ~193 packets).
 - kw planes at 32-aligned partition bases (0/32/64) so the kw0/kw2
   duplicates are built by VECTOR-engine partition-crossing shifted
   copies (nch=24 quadrant moves, DVE 4x bf16 copy).  Pair-boundary
   columns (always zero: the image's L/R padding) are fixed by tiny
   strided memsets; gap rows 27-31/56-63 are zero-weighted, memset once.
 - Statics (indL, indR, ones) at partitions 24:27 fuse bias+border terms.
 - PSUM: 4 x [128,1024] double-bank tiles; two matmuls fill the halves,
   ONE wide drain (vector or scalar) empties both banks - amortizes the
   ~120-170cyc fixed cost and keeps the two engines on DIFFERENT banks
   (PSUM banks are single-ported; v1 split one bank between engines and
   serialized on the port at 512ns/pair).
 - All DMA-visible data is bf16; PSUM stays f32; host casts the output
   back.  Rel err ~3.6e-3 vs the 2e-2 gate.
 - Output: DRAM laid out [chunk=16pairs, 128, 8192] exactly as produced
   -> ONE contiguous SWDGE DMA per chunk (quartered for first/last chunk
   to cut ramp/tail).  Host un-permutes with numpy.
"""

from contextlib import ExitStack

import ml_dtypes
import numpy as np

import concourse.bass as bass
import concourse.tile as tile
from concourse import bacc, mybir
from concourse.bass_utils import run_bass_kernel_spmd

# Problem shapes (hardcoded per contract)
B, CIN, H, W = 8, 3, 512, 512
COUT, E, KS = 64, 3, 3
NCORES = 8
KP = 88            # contraction: kw0 0:24, statics 24:27, 0s 27:32,
                   #              kw1 32:56, 0s 56:64, kw2 64:88
NST = 8            # supertiles
SW = 16 * W        # free elems per partition per supertile (8192 = 16KB bf16)
BF16 = mybir.dt.bfloat16
F32 = mybir.dt.float32
NPBF16 = ml_dtypes.bfloat16

_cache: dict = {}


def _build():
    nchunk = NST * 2          # 16 chunks of 16 pairs (output granularity)
    cw = SW                   # 8192

    nc = bacc.Bacc("TRN2", target_bir_lowering=False, debug=False)
    # rows 0:24 = kw1 data; rows 24:32 = zeros, read only by each buffer's
    # FIRST fill to initialize the zero-weighted gap partitions 56:64
    # (DVE memsets of 8192 elems run at 1x = ~7us each - far too slow).
    xin = nc.dram_tensor("xin", [NST, 32, SW], BF16, kind="ExternalInput").ap()
    wts = nc.dram_tensor("wts", [KP, 4 * 128], BF16, kind="ExternalInput").ap()
    # statics (indL, indR, ones) + 5 zero rows: lands on partitions 24:32
    stat = nc.dram_tensor("stat", [8, SW], BF16, kind="ExternalInput").ap()
    out = nc.dram_tensor("out", [nchunk, 128, cw], BF16, kind="ExternalOutput").ap()

    PBUFS = 4
    OBUFS = 6
    # drains handled by vector for these (g*8+seg//2) indices, scalar else;
    # vector also carries the dups, so scalar takes the bigger drain share.
    # Vector's g1 drains sit EARLY (8, 10) so the dup_b emitted after index
    # 11 leaves only scalar-drained banks behind it (no PE stall while the
    # vector is busy with the 2.4us dup half).
    VDRAIN = {1, 4, 7, 8, 10}

    with tile.TileContext(nc) as tc, ExitStack() as ctx:
        wpool = ctx.enter_context(tc.tile_pool(name="wpool", bufs=1))
        ppool = ctx.enter_context(tc.tile_pool(name="ppool", bufs=PBUFS))
        opool = ctx.enter_context(tc.tile_pool(name="opool", bufs=OBUFS))
        pspool = ctx.enter_context(tc.tile_pool(name="pspool", bufs=4, space="PSUM"))

        # wtile rides the (otherwise idle) sync HWDGE ring: its 88 tiny 1KB
        # descriptors are latency-bound packets and on the gpsimd queue they
        # delayed the first matmul to ~19.5us.
        wtile = wpool.tile([KP, 4 * 128], BF16)
        nc.sync.dma_start(
            wtile[:, :], bass.AP(wts.tensor, 0, [[4 * 128, KP], [1, 4 * 128]])
        )

        patch_tiles = []
        for s in range(PBUFS):
            pt = ppool.tile([KP, SW], BF16, name=f"patch{s}", tag="patch")
            patch_tiles.append(pt)

        def init_buf(s):
            # One-time per physical buffer: the two never-written shift
            # edges, and the statics DMA (rows 24:32 = 3 statics + 5 zeros).
            pt = patch_tiles[s]
            nc.vector.memset(pt[0:24, 0:1], 0.0)
            nc.vector.memset(pt[64:88, SW - 1 : SW], 0.0)
            nc.gpsimd.dma_start(pt[24:32, :], stat[:, :])

        def fill(st):
            # A buffer's first fill loads 32 rows (data + embedded zeros
            # for gap partitions 56:64); later fills reuse the zeros and
            # load only the 24 data rows.
            pt = patch_tiles[st % PBUFS]
            base = st * 32 * SW
            nrow = 32 if st < PBUFS else 24
            # st 0 goes wholly on the scalar ring (sync is busy with wtile).
            eng = (nc.scalar, nc.sync)[st % 2] if st else nc.scalar
            eng.dma_start(
                pt[32 : 32 + nrow, :],
                bass.AP(xin.tensor, base, [[SW, nrow], [1, SW]]),
            )

        def dup_a(st):
            # kw0 plane (= kw1 shifted +1 elem) on the vector engine; even
            # free-dim splits keep DVE 4x mode.  Pair-boundary columns are
            # the image's L zero padding; the shifted copy puts the
            # neighbour pair's edge there - zero them after.
            pt = patch_tiles[st % PBUFS]
            h = SW // 2  # 4096
            nc.vector.tensor_copy(pt[0:24, 1 : h + 1], pt[32:56, 0:h])
            nc.vector.tensor_copy(pt[0:24, h + 1 : SW - 1], pt[32:56, h : SW - 2])
            nc.vector.tensor_copy(pt[0:24, SW - 1 : SW], pt[32:56, SW - 2 : SW - 1])
            nc.vector.memset(pt[0:24, W :: W], 0.0)

        def dup_b(st):
            # kw2 plane (= kw1 shifted -1 elem); R-padding boundary fixups.
            pt = patch_tiles[st % PBUFS]
            h = SW // 2
            nc.vector.tensor_copy(pt[64:88, 0:h], pt[32:56, 1 : h + 1])
            nc.vector.tensor_copy(pt[64:88, h : SW - 2], pt[32:56, h + 1 : SW - 1])
            nc.vector.tensor_copy(pt[64:88, SW - 2 : SW - 1], pt[32:56, SW - 1 : SW])
            nc.vector.memset(pt[64:88, W - 1 : SW - W : W], 0.0)

        init_buf(0)
        init_buf(1)
        fill(0)
        fill(1)
        dup_a(0)
        dup_b(0)

        for st in range(NST):
            pt = patch_tiles[st % PBUFS]
            if st + 2 < NST:
                if st + 2 < PBUFS:
                    init_buf(st + 2)
                fill(st + 2)

            for g in range(2):
                ch = st * 2 + g
                ob = opool.tile([128, cw], BF16, name="ob", tag="ob")
                # Split first/last chunks' output DMA so early pieces drain
                # while later ones compute (shorter ramp/tail); the very
                # last chunk gets eighths to minimize the tail.
                quarters = (
                    8 if ch == nchunk - 1 else (4 if ch in (0, nchunk - 2) else 1)
                )
                for q in range(quarters):
                    s0 = q * 16 // quarters
                    s1 = (q + 1) * 16 // quarters
                    for seg in range(s0, s1):
                        pairidx = st * 32 + g * 16 + seg
                        var = (
                            0
                            if pairidx == 0
                            else (3 if pairidx == 255 else (1 + g))
                        )
                        if seg % 2 == 0:
                            ps2 = pspool.tile([128, 2 * W], F32, name="ps", tag="ps")
                        half = seg % 2
                        nc.tensor.matmul(
                            ps2[:, half * W : (half + 1) * W],
                            wtile[:, var * 128 : (var + 1) * 128],
                            pt[:, seg * W : (seg + 1) * W],
                            start=True,
                            stop=True,
                        )
                        if seg % 2 == 1:
                            dst = ob[:, (seg - 1) * W : (seg + 1) * W]
                            if (g * 8 + seg // 2) in VDRAIN:
                                nc.vector.tensor_copy(dst, ps2[:, :])
                            else:
                                nc.scalar.copy(dst, ps2[:, :])
                            if g == 1 and seg == 7 and st + 1 < NST:
                                dup_b(st + 1)

                    dma_dst = bass.AP(
                        out.tensor,
                        ch * 128 * cw + s0 * W,
                        [[cw, 128], [1, (s1 - s0) * W]],
                    )
                    nc.gpsimd.dma_start(dma_dst, ob[:, s0 * W : s1 * W])

                if g == 0 and st + 1 < NST:
                    # Next supertile's kw0 dup sits between this supertile's
                    # two drain batches in the vector FIFO; the kw2 half is
                    # emitted inside g1 after drain index 11.
                    dup_a(st + 1)

    nc.compile()
    return nc


def _host_prep(x, v, wm, bm, we, be):
    """Per-core inputs: packed kw=1 row-planes (supertile layout), fused
    weights (4 stationary variants), statics."""
    Bb = x.shape[0]
    vr = v.reshape(Bb, COUT, E).astype(np.float64)

    # Border-class table for the "extra" conv of a spatially-constant image:
    # T[b,c,clsh,clsw] = sum of kernel taps landing inside + both biases.
    sets = {0: [1, 2], 1: [0, 1, 2], 2: [0, 1]}
    Mcl = np.zeros((COUT, E, 3, 3), np.float64)
    we64 = we.astype(np.float64)
    for ch_ in range(3):
        for cw_ in range(3):
            Mcl[:, :, ch_, cw_] = we64[:, :, sets[ch_], :][:, :, :, sets[cw_]].sum((2, 3))
    T = (
        np.einsum("bce,cehw->bchw", vr, Mcl)
        + bm.astype(np.float64)[None, :, None, None]
        + be.astype(np.float64)[None, :, None, None]
    )

    # xin[b, st, (d*3+ci)*2+g, seg*512+y] = xr[b, ci, 2*(st*32+g*16+seg)+d, y]
    xr = np.pad(x, ((0, 0), (0, 0), (1, 1), (0, 0))).astype(np.float32)
    xin5 = np.zeros((Bb, NST, 16, 2, 16, W), np.float32)
    for d in range(4):
        sl = xr[:, :, d : d + 511 : 2, :]  # rows d, d+2, ..., d+510 -> 256
        xin5[:, :, d * 3 : (d + 1) * 3] = sl.reshape(
            Bb, CIN, NST, 2, 16, W
        ).transpose(0, 2, 1, 3, 4, 5)
    xin = xin5.reshape(Bb, NST, 32, SW).astype(NPBF16)

    # Stationary variants: (vrow, g) in [(0,0),(1,0),(1,1),(2,1)].
    # vrow: 0 = pair 0 (rows top,mid); 1 = interior; 2 = last pair (mid,bot)
    pair_cls = {0: (0, 1), 1: (1, 1), 2: (1, 2)}
    plane_base = {0: 0, 1: 32, 2: 64}
    var_map = [(0, 0), (1, 0), (1, 1), (2, 1)]
    wts = np.zeros((Bb, 4, KP, 128), np.float32)
    for b in range(Bb):
        for var, (vrow, gsel) in enumerate(var_map):
            for p in range(2):
                cols = slice(p * 64, p * 64 + 64)
                for kw in range(KS):
                    for d in range(4):
                        kh = d - p
                        if 0 <= kh < KS:
                            for ci in range(CIN):
                                k = plane_base[kw] + (d * 3 + ci) * 2 + gsel
                                wts[b, var, k, cols] = wm[:, ci, kh, kw]
                cls = pair_cls[vrow][p]
                wts[b, var, 24, cols] = T[b, :, cls, 0] - T[b, :, cls, 1]
                wts[b, var, 25, cols] = T[b, :, cls, 2] - T[b, :, cls, 1]
                wts[b, var, 26, cols] = T[b, :, cls, 1]

    # DRAM layout = wtile layout: wts2[b, k, var*128+m] = wts[b, var, k, m]
    wts2 = np.ascontiguousarray(wts.transpose(0, 2, 1, 3)).reshape(Bb, KP, 4 * 128)

    stat = np.zeros((8, SW), np.float32)
    stat[0, 0::W] = 1.0            # output col 0 (left border class)
    stat[1, W - 1 :: W] = 1.0      # output col w-1 (right border class)
    stat[2, :] = 1.0               # ones row (base bias + interior class)
    return xin, wts2.astype(NPBF16), stat.astype(NPBF16)


def _unpack_out(o, h=H, w=W, c=16):
    """[nchunk, 128, c*w] -> [COUT, h, w]; partition = p*64+co,
    free = seg*w+x, row = ch*2c + 2*seg + p."""
    nchunk = (h // 2) // c
    return (
        o.reshape(nchunk, 2, COUT, c, w)
        .transpose(2, 0, 3, 1, 4)
        .reshape(COUT, h, w)
    )


def kernel(**inputs) -> np.ndarray:
    x = np.ascontiguousarray(np.asarray(inputs["x"], np.float32))
    v = np.asarray(inputs["extra_inputs"], np.float32)
    wm = np.asarray(inputs["w_main"], np.float32)
    bm = np.asarray(inputs["b_main"], np.float32)
    we = np.asarray(inputs["w_extra"], np.float32)
    be = np.asarray(inputs["b_extra"], np.float32)

    xin, wts, stat = _host_prep(x, v, wm, bm, we, be)

    if "nc" not in _cache:
        _cache["nc"] = _build()
    nc = _cache["nc"]

    in_maps = [{"xin": xin[b], "wts": wts[b], "stat": stat} for b in range(B)]
    res = run_bass_kernel_spmd(nc, in_maps, list(range(NCORES)))
    return np.stack(
        [_unpack_out(res.results[b]["out"]) for b in range(B)]
    ).astype(np.float32)


# revision 27
# speedup vs baseline: 1.0067x; 1.0067x over previous
"""Trainium2 Bass kernel for CustomConvWithExtra.

out = conv3x3(x, w_main) + b_main + extra, where extra collapses to a 3x3
border-class table T[b,c,clsh,clsw] (conv of a spatially-constant image).

Design v3 (from 183us baseline; DMA packet-latency is the binding law):
 - Data parallel: 1 batch image per NeuronCore (B=8 = 8 cores).
 - Empirical DMA law (measured): ANY packet that WRITES SBUF costs ~1.05us
   flat (HBM round-trip latency, no pipelining, packet <= 16KB); only
   SBUF->HBM writes stream at ~26GB/s/engine.  So the input path must
   minimize SBUF-write packet count and the kw=0/2 duplicate planes must
   NOT go through DMA at all.
 - Supertile = 32 output row-pairs; 8 supertiles.  Patch tile [88, 8192]
   bf16: each kw plane holds 24 rows = (d,ci) x (g=half), each row 16
   pair-segments of 512 = 16KB -> fill is 24 descriptors of EXACTLY 16KB
   (one full packet each; 3.2MB total input = ~193 packets).
 - kw planes at 32-aligned partition bases (0/32/64) so the kw0/kw2
   duplicates are built by VECTOR-engine partition-crossing shifted
   copies (nch=24 quadrant moves, DVE 4x bf16 copy).  Pair-boundary
   columns (always zero: the image's L/R padding) are fixed by tiny
   strided memsets; gap rows 27-31/56-63 are zero-weighted, memset once.
 - Statics (indL, indR, ones) at partitions 24:27 fuse bias+border terms.
 - PSUM: 4 x [128,1024] double-bank tiles; two matmuls fill the halves,
   ONE wide drain (vector or scalar) empties both banks - amortizes the
   ~120-170cyc fixed cost and keeps the two engines on DIFFERENT banks
   (PSUM banks are single-ported; v1 split one bank between engines and
   serialized on the port at 512ns/pair).
 - All DMA-visible data is bf16; PSUM stays f32; host casts the output
   back.  Rel err ~3.6e-3 vs the 2e-2 gate.
 - Output: DRAM laid out [chunk=16pairs, 128, 8192] exactly as produced
   -> ONE contiguous SWDGE DMA per chunk (quartered for first/last chunk
   to cut ramp/tail).  Host un-permutes with numpy.
"""

from contextlib import ExitStack

import ml_dtypes
import numpy as np

import concourse.bass as bass
import concourse.tile as tile
from concourse import bacc, mybir
from concourse.bass_utils import run_bass_kernel_spmd

# Problem shapes (hardcoded per contract)
B, CIN, H, W = 8, 3, 512, 512
COUT, E, KS = 64, 3, 3
NCORES = 8
KP = 88            # contraction: kw0 0:24, statics 24:27, 0s 27:32,
                   #              kw1 32:56, 0s 56:64, kw2 64:88
NST = 8            # supertiles
SW = 16 * W        # free elems per partition per supertile (8192 = 16KB bf16)
BF16 = mybir.dt.bfloat16
F32 = mybir.dt.float32
NPBF16 = ml_dtypes.bfloat16

_cache: dict = {}


def _build():
    nchunk = NST * 2          # 16 chunks of 16 pairs (output granularity)
    cw = SW                   # 8192

    nc = bacc.Bacc("TRN2", target_bir_lowering=False, debug=False)
    # rows 0:24 = kw1 data; rows 24:32 = zeros, read only by each buffer's
    # FIRST fill to initialize the zero-weighted gap partitions 56:64
    # (DVE memsets of 8192 elems run at 1x = ~7us each - far too slow).
    xin = nc.dram_tensor("xin", [NST, 32, SW], BF16, kind="ExternalInput").ap()
    wts = nc.dram_tensor("wts", [KP, 4 * 128], BF16, kind="ExternalInput").ap()
    # statics (indL, indR, ones) + 5 zero rows: lands on partitions 24:32
    stat = nc.dram_tensor("stat", [8, SW], BF16, kind="ExternalInput").ap()
    out = nc.dram_tensor("out", [nchunk, 128, cw], BF16, kind="ExternalOutput").ap()

    PBUFS = 4
    OBUFS = 6
    # drains handled by vector for these (g*8+seg//2) indices, scalar else;
    # vector also carries the dups, so scalar takes the bigger drain share.
    # Vector's g1 drains sit EARLY (8, 10) so the dup_b emitted after index
    # 11 leaves only scalar-drained banks behind it (no PE stall while the
    # vector is busy with the 2.4us dup half).
    VDRAIN = {1, 4, 7, 8, 10}

    with tile.TileContext(nc) as tc, ExitStack() as ctx:
        wpool = ctx.enter_context(tc.tile_pool(name="wpool", bufs=1))
        ppool = ctx.enter_context(tc.tile_pool(name="ppool", bufs=PBUFS))
        opool = ctx.enter_context(tc.tile_pool(name="opool", bufs=OBUFS))
        pspool = ctx.enter_context(tc.tile_pool(name="pspool", bufs=4, space="PSUM"))

        wtile = wpool.tile([KP, 4 * 128], BF16)

        def load_wtile():
            # wtile's 88 tiny 1KB descriptors are latency-bound packets
            # (~1.5us each); split the load across two rings so the halves
            # land in parallel (~6us instead of ~12).
            nc.sync.dma_start(
                wtile[0:44, :], bass.AP(wts.tensor, 0, [[4 * 128, 44], [1, 4 * 128]])
            )
            nc.gpsimd.dma_start(
                wtile[44:88, :],
                bass.AP(wts.tensor, 44 * 4 * 128, [[4 * 128, 44], [1, 4 * 128]]),
            )

        patch_tiles = []
        for s in range(PBUFS):
            pt = ppool.tile([KP, SW], BF16, name=f"patch{s}", tag="patch")
            patch_tiles.append(pt)

        def init_buf(s):
            # One-time per physical buffer: the two never-written shift
            # edges, and the statics DMA (rows 24:32 = 3 statics + 5 zeros).
            pt = patch_tiles[s]
            nc.vector.memset(pt[0:24, 0:1], 0.0)
            nc.vector.memset(pt[64:88, SW - 1 : SW], 0.0)
            nc.gpsimd.dma_start(pt[24:32, :], stat[:, :])

        def fill(st):
            # A buffer's first fill loads 32 rows (data + embedded zeros
            # for gap partitions 56:64); later fills reuse the zeros and
            # load only the 24 data rows.
            pt = patch_tiles[st % PBUFS]
            base = st * 32 * SW
            nrow = 32 if st < PBUFS else 24
            # st 0 goes wholly on the scalar ring (sync is busy with wtile).
            eng = (nc.scalar, nc.sync)[st % 2] if st else nc.scalar
            eng.dma_start(
                pt[32 : 32 + nrow, :],
                bass.AP(xin.tensor, base, [[SW, nrow], [1, SW]]),
            )

        def dup_a(st):
            # kw0 plane (= kw1 shifted +1 elem) on the vector engine; even
            # free-dim splits keep DVE 4x mode.  Pair-boundary columns are
            # the image's L zero padding; the shifted copy puts the
            # neighbour pair's edge there - zero them after.
            pt = patch_tiles[st % PBUFS]
            h = SW // 2  # 4096
            nc.vector.tensor_copy(pt[0:24, 1 : h + 1], pt[32:56, 0:h])
            nc.vector.tensor_copy(pt[0:24, h + 1 : SW - 1], pt[32:56, h : SW - 2])
            nc.vector.tensor_copy(pt[0:24, SW - 1 : SW], pt[32:56, SW - 2 : SW - 1])
            nc.vector.memset(pt[0:24, W :: W], 0.0)

        def dup_b(st):
            # kw2 plane (= kw1 shifted -1 elem); R-padding boundary fixups.
            pt = patch_tiles[st % PBUFS]
            h = SW // 2
            nc.vector.tensor_copy(pt[64:88, 0:h], pt[32:56, 1 : h + 1])
            nc.vector.tensor_copy(pt[64:88, h : SW - 2], pt[32:56, h + 1 : SW - 1])
            nc.vector.tensor_copy(pt[64:88, SW - 2 : SW - 1], pt[32:56, SW - 1 : SW])
            nc.vector.memset(pt[64:88, W - 1 : SW - W : W], 0.0)

        # Ramp order: statics first on gpsimd, fill(0) on scalar, wtile
        # halves on sync+gpsimd, fill(1) behind the sync half.
        init_buf(0)
        init_buf(1)
        fill(0)
        load_wtile()
        fill(1)
        dup_a(0)
        dup_b(0)

        for st in range(NST):
            pt = patch_tiles[st % PBUFS]
            if st + 2 < NST:
                if st + 2 < PBUFS:
                    init_buf(st + 2)
                fill(st + 2)

            for g in range(2):
                ch = st * 2 + g
                ob = opool.tile([128, cw], BF16, name="ob", tag="ob")
                # Split first/last chunks' output DMA so early pieces drain
                # while later ones compute (shorter ramp/tail).
                quarters = 4 if ch in (0, nchunk - 2, nchunk - 1) else 1
                for q in range(quarters):
                    s0 = q * 16 // quarters
                    s1 = (q + 1) * 16 // quarters
                    for seg in range(s0, s1):
                        pairidx = st * 32 + g * 16 + seg
                        var = (
                            0
                            if pairidx == 0
                            else (3 if pairidx == 255 else (1 + g))
                        )
                        if seg % 2 == 0:
                            ps2 = pspool.tile([128, 2 * W], F32, name="ps", tag="ps")
                        half = seg % 2
                        nc.tensor.matmul(
                            ps2[:, half * W : (half + 1) * W],
                            wtile[:, var * 128 : (var + 1) * 128],
                            pt[:, seg * W : (seg + 1) * W],
                            start=True,
                            stop=True,
                        )
                        if seg % 2 == 1:
                            dst = ob[:, (seg - 1) * W : (seg + 1) * W]
                            if (g * 8 + seg // 2) in VDRAIN:
                                nc.vector.tensor_copy(dst, ps2[:, :])
                            else:
                                nc.scalar.copy(dst, ps2[:, :])


                    dma_dst = bass.AP(
                        out.tensor,
                        ch * 128 * cw + s0 * W,
                        [[cw, 128], [1, (s1 - s0) * W]],
                    )
                    nc.gpsimd.dma_start(dma_dst, ob[:, s0 * W : s1 * W])

                if g == 0 and st + 1 < NST:
                    # Next supertile's dups, DEMOTED ~30 priority slots: the
                    # Tile scheduler then prefers this supertile's g1 drains
                    # (which pace the PE via PSUM bank-free) and slots the
                    # dup pieces into the vector's idle gaps.
                    with tc.high_priority(offset=-30):
                        dup_a(st + 1)
                        dup_b(st + 1)

    nc.compile()
    return nc


def _host_prep(x, v, wm, bm, we, be):
    """Per-core inputs: packed kw=1 row-planes (supertile layout), fused
    weights (4 stationary variants), statics."""
    Bb = x.shape[0]
    vr = v.reshape(Bb, COUT, E).astype(np.float64)

    # Border-class table for the "extra" conv of a spatially-constant image:
    # T[b,c,clsh,clsw] = sum of kernel taps landing inside + both biases.
    sets = {0: [1, 2], 1: [0, 1, 2], 2: [0, 1]}
    Mcl = np.zeros((COUT, E, 3, 3), np.float64)
    we64 = we.astype(np.float64)
    for ch_ in range(3):
        for cw_ in range(3):
            Mcl[:, :, ch_, cw_] = we64[:, :, sets[ch_], :][:, :, :, sets[cw_]].sum((2, 3))
    T = (
        np.einsum("bce,cehw->bchw", vr, Mcl)
        + bm.astype(np.float64)[None, :, None, None]
        + be.astype(np.float64)[None, :, None, None]
    )

    # xin[b, st, (d*3+ci)*2+g, seg*512+y] = xr[b, ci, 2*(st*32+g*16+seg)+d, y]
    xr = np.pad(x, ((0, 0), (0, 0), (1, 1), (0, 0))).astype(np.float32)
    xin5 = np.zeros((Bb, NST, 16, 2, 16, W), np.float32)
    for d in range(4):
        sl = xr[:, :, d : d + 511 : 2, :]  # rows d, d+2, ..., d+510 -> 256
        xin5[:, :, d * 3 : (d + 1) * 3] = sl.reshape(
            Bb, CIN, NST, 2, 16, W
        ).transpose(0, 2, 1, 3, 4, 5)
    xin = xin5.reshape(Bb, NST, 32, SW).astype(NPBF16)

    # Stationary variants: (vrow, g) in [(0,0),(1,0),(1,1),(2,1)].
    # vrow: 0 = pair 0 (rows top,mid); 1 = interior; 2 = last pair (mid,bot)
    pair_cls = {0: (0, 1), 1: (1, 1), 2: (1, 2)}
    plane_base = {0: 0, 1: 32, 2: 64}
    var_map = [(0, 0), (1, 0), (1, 1), (2, 1)]
    wts = np.zeros((Bb, 4, KP, 128), np.float32)
    for b in range(Bb):
        for var, (vrow, gsel) in enumerate(var_map):
            for p in range(2):
                cols = slice(p * 64, p * 64 + 64)
                for kw in range(KS):
                    for d in range(4):
                        kh = d - p
                        if 0 <= kh < KS:
                            for ci in range(CIN):
                                k = plane_base[kw] + (d * 3 + ci) * 2 + gsel
                                wts[b, var, k, cols] = wm[:, ci, kh, kw]
                cls = pair_cls[vrow][p]
                wts[b, var, 24, cols] = T[b, :, cls, 0] - T[b, :, cls, 1]
                wts[b, var, 25, cols] = T[b, :, cls, 2] - T[b, :, cls, 1]
                wts[b, var, 26, cols] = T[b, :, cls, 1]

    # DRAM layout = wtile layout: wts2[b, k, var*128+m] = wts[b, var, k, m]
    wts2 = np.ascontiguousarray(wts.transpose(0, 2, 1, 3)).reshape(Bb, KP, 4 * 128)

    stat = np.zeros((8, SW), np.float32)
    stat[0, 0::W] = 1.0            # output col 0 (left border class)
    stat[1, W - 1 :: W] = 1.0      # output col w-1 (right border class)
    stat[2, :] = 1.0               # ones row (base bias + interior class)
    return xin, wts2.astype(NPBF16), stat.astype(NPBF16)


def _unpack_out(o, h=H, w=W, c=16):
    """[nchunk, 128, c*w] -> [COUT, h, w]; partition = p*64+co,
    free = seg*w+x, row = ch*2c + 2*seg + p."""
    nchunk = (h // 2) // c
    return (
        o.reshape(nchunk, 2, COUT, c, w)
        .transpose(2, 0, 3, 1, 4)
        .reshape(COUT, h, w)
    )


def kernel(**inputs) -> np.ndarray:
    x = np.ascontiguousarray(np.asarray(inputs["x"], np.float32))
    v = np.asarray(inputs["extra_inputs"], np.float32)
    wm = np.asarray(inputs["w_main"], np.float32)
    bm = np.asarray(inputs["b_main"], np.float32)
    we = np.asarray(inputs["w_extra"], np.float32)
    be = np.asarray(inputs["b_extra"], np.float32)

    xin, wts, stat = _host_prep(x, v, wm, bm, we, be)

    if "nc" not in _cache:
        _cache["nc"] = _build()
    nc = _cache["nc"]

    in_maps = [{"xin": xin[b], "wts": wts[b], "stat": stat} for b in range(B)]
    res = run_bass_kernel_spmd(nc, in_maps, list(range(NCORES)))
    return np.stack(
        [_unpack_out(res.results[b]["out"]) for b in range(B)]
    ).astype(np.float32)


# revision 30
# speedup vs baseline: 1.0119x; 1.0052x over previous
"""Trainium2 Bass kernel for CustomConvWithExtra.

out = conv3x3(x, w_main) + b_main + extra, where extra collapses to a 3x3
border-class table T[b,c,clsh,clsw] (conv of a spatially-constant image).

Design v3 (from 183us baseline; DMA packet-latency is the binding law):
 - Data parallel: 1 batch image per NeuronCore (B=8 = 8 cores).
 - Empirical DMA law (measured): ANY packet that WRITES SBUF costs ~1.05us
   flat (HBM round-trip latency, no pipelining, packet <= 16KB); only
   SBUF->HBM writes stream at ~26GB/s/engine.  So the input path must
   minimize SBUF-write packet count and the kw=0/2 duplicate planes must
   NOT go through DMA at all.
 - Supertile = 32 output row-pairs; 8 supertiles.  Patch tile [88, 8192]
   bf16: each kw plane holds 24 rows = (d,ci) x (g=half), each row 16
   pair-segments of 512 = 16KB -> fill is 24 descriptors of EXACTLY 16KB
   (one full packet each; 3.2MB total input = ~193 packets).
 - kw planes at 32-aligned partition bases (0/32/64) so the kw0/kw2
   duplicates are built by VECTOR-engine partition-crossing shifted
   copies (nch=24 quadrant moves, DVE 4x bf16 copy).  Pair-boundary
   columns (always zero: the image's L/R padding) are fixed by tiny
   strided memsets; gap rows 27-31/56-63 are zero-weighted, memset once.
 - Statics (indL, indR, ones) at partitions 24:27 fuse bias+border terms.
 - PSUM: 4 x [128,1024] double-bank tiles; two matmuls fill the halves,
   ONE wide drain (vector or scalar) empties both banks - amortizes the
   ~120-170cyc fixed cost and keeps the two engines on DIFFERENT banks
   (PSUM banks are single-ported; v1 split one bank between engines and
   serialized on the port at 512ns/pair).
 - All DMA-visible data is bf16; PSUM stays f32; host casts the output
   back.  Rel err ~3.6e-3 vs the 2e-2 gate.
 - Output: DRAM laid out [chunk=16pairs, 128, 8192] exactly as produced
   -> ONE contiguous SWDGE DMA per chunk (quartered for first/last chunk
   to cut ramp/tail).  Host un-permutes with numpy.
"""

from contextlib import ExitStack

import ml_dtypes
import numpy as np

import concourse.bass as bass
import concourse.tile as tile
from concourse import bacc, mybir
from concourse.bass_utils import run_bass_kernel_spmd

# Problem shapes (hardcoded per contract)
B, CIN, H, W = 8, 3, 512, 512
COUT, E, KS = 64, 3, 3
NCORES = 8
KP = 88            # contraction: kw0 0:24, statics 24:27, 0s 27:32,
                   #              kw1 32:56, 0s 56:64, kw2 64:88
NST = 8            # supertiles
SW = 16 * W        # free elems per partition per supertile (8192 = 16KB bf16)
BF16 = mybir.dt.bfloat16
F32 = mybir.dt.float32
NPBF16 = ml_dtypes.bfloat16

_cache: dict = {}


def _build():
    nchunk = NST * 2          # 16 chunks of 16 pairs (output granularity)
    cw = SW                   # 8192

    nc = bacc.Bacc("TRN2", target_bir_lowering=False, debug=False)
    # rows 0:24 = kw1 data; rows 24:32 = zeros, read only by each buffer's
    # FIRST fill to initialize the zero-weighted gap partitions 56:64
    # (DVE memsets of 8192 elems run at 1x = ~7us each - far too slow).
    xin = nc.dram_tensor("xin", [NST, 32, SW], BF16, kind="ExternalInput").ap()
    wts = nc.dram_tensor("wts", [KP, 4 * 128], BF16, kind="ExternalInput").ap()
    # statics (indL, indR, ones) + 5 zero rows: lands on partitions 24:32
    stat = nc.dram_tensor("stat", [8, SW], BF16, kind="ExternalInput").ap()
    out = nc.dram_tensor("out", [nchunk, 128, cw], BF16, kind="ExternalOutput").ap()

    PBUFS = 4
    OBUFS = 6
    # drains handled by vector for these (g*8+seg//2) indices, scalar else;
    # vector also carries the dups, so scalar takes the bigger drain share.
    # Striped every 3rd: scalar never runs more than 2 drains back-to-back,
    # so no PSUM bank-free ever lags a long scalar queue (the {1,4,7,8,10}
    # variant left drains 11-15 all-scalar and the next chunk's MMs stalled
    # ~1.2us at seg 6/10/12 every supertile).
    VDRAIN = {2, 5, 8, 11, 14}

    with tile.TileContext(nc) as tc, ExitStack() as ctx:
        wpool = ctx.enter_context(tc.tile_pool(name="wpool", bufs=1))
        ppool = ctx.enter_context(tc.tile_pool(name="ppool", bufs=PBUFS))
        opool = ctx.enter_context(tc.tile_pool(name="opool", bufs=OBUFS))
        pspool = ctx.enter_context(tc.tile_pool(name="pspool", bufs=4, space="PSUM"))

        wtile = wpool.tile([KP, 4 * 128], BF16)

        def load_wtile():
            # wtile's 88 tiny 1KB descriptors are latency-bound packets
            # (~2us+ each); split the load across all THREE queues so the
            # thirds land in parallel, each behind only MM0-gating work.
            for eng, lo, hi in (
                (nc.sync, 0, 30),
                (nc.scalar, 30, 59),
                (nc.gpsimd, 59, 88),
            ):
                eng.dma_start(
                    wtile[lo:hi, :],
                    bass.AP(
                        wts.tensor, lo * 4 * 128, [[4 * 128, hi - lo], [1, 4 * 128]]
                    ),
                )

        patch_tiles = []
        for s in range(PBUFS):
            pt = ppool.tile([KP, SW], BF16, name=f"patch{s}", tag="patch")
            patch_tiles.append(pt)

        def init_buf(s):
            # One-time per physical buffer: the two never-written shift
            # edges, and the statics DMA (rows 24:32 = 3 statics + 5 zeros).
            pt = patch_tiles[s]
            nc.vector.memset(pt[0:24, 0:1], 0.0)
            nc.vector.memset(pt[64:88, SW - 1 : SW], 0.0)
            nc.gpsimd.dma_start(pt[24:32, :], stat[:, :])

        def fill(st):
            # A buffer's first fill loads 32 rows (data + embedded zeros
            # for gap partitions 56:64); later fills reuse the zeros and
            # load only the 24 data rows.
            pt = patch_tiles[st % PBUFS]
            base = st * 32 * SW
            nrow = 32 if st < PBUFS else 24
            # st 0 goes wholly on the scalar ring (sync is busy with wtile).
            eng = (nc.scalar, nc.sync)[st % 2] if st else nc.scalar
            eng.dma_start(
                pt[32 : 32 + nrow, :],
                bass.AP(xin.tensor, base, [[SW, nrow], [1, SW]]),
            )

        def dup_a(st):
            # kw0 plane (= kw1 shifted +1 elem) on the vector engine; even
            # free-dim splits keep DVE 4x mode.  Pair-boundary columns are
            # the image's L zero padding; the shifted copy puts the
            # neighbour pair's edge there - zero them after.
            pt = patch_tiles[st % PBUFS]
            h = SW // 2  # 4096
            nc.vector.tensor_copy(pt[0:24, 1 : h + 1], pt[32:56, 0:h])
            nc.vector.tensor_copy(pt[0:24, h + 1 : SW - 1], pt[32:56, h : SW - 2])
            nc.vector.tensor_copy(pt[0:24, SW - 1 : SW], pt[32:56, SW - 2 : SW - 1])
            nc.vector.memset(pt[0:24, W :: W], 0.0)

        def dup_b(st):
            # kw2 plane (= kw1 shifted -1 elem); R-padding boundary fixups.
            pt = patch_tiles[st % PBUFS]
            h = SW // 2
            nc.vector.tensor_copy(pt[64:88, 0:h], pt[32:56, 1 : h + 1])
            nc.vector.tensor_copy(pt[64:88, h : SW - 2], pt[32:56, h + 1 : SW - 1])
            nc.vector.tensor_copy(pt[64:88, SW - 2 : SW - 1], pt[32:56, SW - 1 : SW])
            nc.vector.memset(pt[64:88, W - 1 : SW - W : W], 0.0)

        # Per-ring ramp order (everything before wtile gates MM0 anyway):
        #   sync:   wtile/3, fill(1)        scalar: fill(0), wtile/3
        #   gpsimd: stat(0), wtile/3, stat(1)
        init_buf(0)
        fill(0)
        load_wtile()
        init_buf(1)
        fill(1)
        dup_a(0)
        dup_b(0)

        for st in range(NST):
            pt = patch_tiles[st % PBUFS]
            if st + 2 < NST:
                if st + 2 < PBUFS:
                    init_buf(st + 2)
                fill(st + 2)

            for g in range(2):
                ch = st * 2 + g
                ob = opool.tile([128, cw], BF16, name="ob", tag="ob")
                # Split first/last chunks' output DMA so early pieces drain
                # while later ones compute (shorter ramp/tail).
                quarters = 4 if ch in (0, nchunk - 2, nchunk - 1) else 1
                for q in range(quarters):
                    s0 = q * 16 // quarters
                    s1 = (q + 1) * 16 // quarters
                    for seg in range(s0, s1):
                        pairidx = st * 32 + g * 16 + seg
                        var = (
                            0
                            if pairidx == 0
                            else (3 if pairidx == 255 else (1 + g))
                        )
                        if seg % 2 == 0:
                            ps2 = pspool.tile([128, 2 * W], F32, name="ps", tag="ps")
                        half = seg % 2
                        nc.tensor.matmul(
                            ps2[:, half * W : (half + 1) * W],
                            wtile[:, var * 128 : (var + 1) * 128],
                            pt[:, seg * W : (seg + 1) * W],
                            start=True,
                            stop=True,
                        )
                        if seg % 2 == 1:
                            dst = ob[:, (seg - 1) * W : (seg + 1) * W]
                            if (g * 8 + seg // 2) in VDRAIN:
                                nc.vector.tensor_copy(dst, ps2[:, :])
                            else:
                                nc.scalar.copy(dst, ps2[:, :])


                    dma_dst = bass.AP(
                        out.tensor,
                        ch * 128 * cw + s0 * W,
                        [[cw, 128], [1, (s1 - s0) * W]],
                    )
                    nc.gpsimd.dma_start(dma_dst, ob[:, s0 * W : s1 * W])

                if g == 0 and st + 1 < NST:
                    # Next supertile's dups, DEMOTED ~30 priority slots: the
                    # Tile scheduler then prefers this supertile's g1 drains
                    # (which pace the PE via PSUM bank-free) and slots the
                    # dup pieces into the vector's idle gaps.
                    with tc.high_priority(offset=-30):
                        dup_a(st + 1)
                        dup_b(st + 1)

    nc.compile()
    return nc


def _host_prep(x, v, wm, bm, we, be):
    """Per-core inputs: packed kw=1 row-planes (supertile layout), fused
    weights (4 stationary variants), statics."""
    Bb = x.shape[0]
    vr = v.reshape(Bb, COUT, E).astype(np.float64)

    # Border-class table for the "extra" conv of a spatially-constant image:
    # T[b,c,clsh,clsw] = sum of kernel taps landing inside + both biases.
    sets = {0: [1, 2], 1: [0, 1, 2], 2: [0, 1]}
    Mcl = np.zeros((COUT, E, 3, 3), np.float64)
    we64 = we.astype(np.float64)
    for ch_ in range(3):
        for cw_ in range(3):
            Mcl[:, :, ch_, cw_] = we64[:, :, sets[ch_], :][:, :, :, sets[cw_]].sum((2, 3))
    T = (
        np.einsum("bce,cehw->bchw", vr, Mcl)
        + bm.astype(np.float64)[None, :, None, None]
        + be.astype(np.float64)[None, :, None, None]
    )

    # xin[b, st, (d*3+ci)*2+g, seg*512+y] = xr[b, ci, 2*(st*32+g*16+seg)+d, y]
    xr = np.pad(x, ((0, 0), (0, 0), (1, 1), (0, 0))).astype(np.float32)
    xin5 = np.zeros((Bb, NST, 16, 2, 16, W), np.float32)
    for d in range(4):
        sl = xr[:, :, d : d + 511 : 2, :]  # rows d, d+2, ..., d+510 -> 256
        xin5[:, :, d * 3 : (d + 1) * 3] = sl.reshape(
            Bb, CIN, NST, 2, 16, W
        ).transpose(0, 2, 1, 3, 4, 5)
    xin = xin5.reshape(Bb, NST, 32, SW).astype(NPBF16)

    # Stationary variants: (vrow, g) in [(0,0),(1,0),(1,1),(2,1)].
    # vrow: 0 = pair 0 (rows top,mid); 1 = interior; 2 = last pair (mid,bot)
    pair_cls = {0: (0, 1), 1: (1, 1), 2: (1, 2)}
    plane_base = {0: 0, 1: 32, 2: 64}
    var_map = [(0, 0), (1, 0), (1, 1), (2, 1)]
    wts = np.zeros((Bb, 4, KP, 128), np.float32)
    for b in range(Bb):
        for var, (vrow, gsel) in enumerate(var_map):
            for p in range(2):
                cols = slice(p * 64, p * 64 + 64)
                for kw in range(KS):
                    for d in range(4):
                        kh = d - p
                        if 0 <= kh < KS:
                            for ci in range(CIN):
                                k = plane_base[kw] + (d * 3 + ci) * 2 + gsel
                                wts[b, var, k, cols] = wm[:, ci, kh, kw]
                cls = pair_cls[vrow][p]
                wts[b, var, 24, cols] = T[b, :, cls, 0] - T[b, :, cls, 1]
                wts[b, var, 25, cols] = T[b, :, cls, 2] - T[b, :, cls, 1]
                wts[b, var, 26, cols] = T[b, :, cls, 1]

    # DRAM layout = wtile layout: wts2[b, k, var*128+m] = wts[b, var, k, m]
    wts2 = np.ascontiguousarray(wts.transpose(0, 2, 1, 3)).reshape(Bb, KP, 4 * 128)

    stat = np.zeros((8, SW), np.float32)
    stat[0, 0::W] = 1.0            # output col 0 (left border class)
    stat[1, W - 1 :: W] = 1.0      # output col w-1 (right border class)
    stat[2, :] = 1.0               # ones row (base bias + interior class)
    return xin, wts2.astype(NPBF16), stat.astype(NPBF16)


def _unpack_out(o, h=H, w=W, c=16):
    """[nchunk, 128, c*w] -> [COUT, h, w]; partition = p*64+co,
    free = seg*w+x, row = ch*2c + 2*seg + p."""
    nchunk = (h // 2) // c
    return (
        o.reshape(nchunk, 2, COUT, c, w)
        .transpose(2, 0, 3, 1, 4)
        .reshape(COUT, h, w)
    )


def kernel(**inputs) -> np.ndarray:
    x = np.ascontiguousarray(np.asarray(inputs["x"], np.float32))
    v = np.asarray(inputs["extra_inputs"], np.float32)
    wm = np.asarray(inputs["w_main"], np.float32)
    bm = np.asarray(inputs["b_main"], np.float32)
    we = np.asarray(inputs["w_extra"], np.float32)
    be = np.asarray(inputs["b_extra"], np.float32)

    xin, wts, stat = _host_prep(x, v, wm, bm, we, be)

    if "nc" not in _cache:
        _cache["nc"] = _build()
    nc = _cache["nc"]

    in_maps = [{"xin": xin[b], "wts": wts[b], "stat": stat} for b in range(B)]
    res = run_bass_kernel_spmd(nc, in_maps, list(range(NCORES)))
    return np.stack(
        [_unpack_out(res.results[b]["out"]) for b in range(B)]
    ).astype(np.float32)
